# revision 10
# baseline (speedup 1.0000x reference)
"""Trainium2 Bass kernel for the interval-prediction custom loss (v5).

total = 10*mean((t - c)^2) + 0.1*mean(u-l) + 10*mean(relu(l-u))
        + 0.5*sum(where(pv==0, relu(c-p), relu(p-c)))/N       with c=(l+u)/2

Host-side prep is layout/encoding only (shard, dtype cast, sign-fold):
  * v-fold: rows with pv==1 are re-encoded (l,u,t,p) -> (-u,-l,-t,-p).
    Exact and information-preserving: l-u and (t-c)^2 are invariant and the
    direction term relu(p-c) becomes relu(c'-p'), so the int64 pv stream
    (unsupported on device) disappears along with its select/mul ops.
  * bf16 streams per element: lo, up, tm2=-2t, pm2=-2p.

Device (per core, [128, 8192] elements), all DVE ops in 2x mode:
  per tile j (width fd): one DMA [P, 4, fd];
     h  = lo+up                       (TT)
     wm_full[j] = lo-up               (TT)
     ex_full[:, :, j] = tp + h_bcast  (ONE TT over [P,2,fd], h stride-0)
       -> e = h+tm2 = l+u-2t, x = h+pm2 = 2*sigma*(c-p)
  PE: psA += ones^T @ wm chunks (width loss partial sums, idle engine)
  per GROUP (amortizes the ~1.3us/op ACT fixed cost; trailing groups are
  single tiles in 'pe' mode so the post-DMA tail stays thin):
     ACT Square(e span, scale=.5) accum ; ACT Relu(wm span) accum
     'act': ACT Relu(x span, scale=.5) accum
     'pe' : DVE TS rd = max(.5x, 0) (4x) ; PE psD += ones^T @ rd chunks
Host combines group accumulators + the two PSUM rows in f64.
"""

import sys

if "/opt/trn_rl_repo" not in sys.path:
    sys.path.insert(0, "/opt/trn_rl_repo")

import numpy as np

N = 8388608
N_CORES = 8
P = 128
NP_PER_CORE = N // N_CORES            # 1048576
FPL = NP_PER_CORE // P                # 8192 elements per partition lane
TILE_WIDTHS = (512, 1024, 2048, 2048, 2048, 512)
assert sum(TILE_WIDTHS) == FPL
# Groups of consecutive tiles sharing one set of ACT ops; dmode 'act' or 'pe'
GROUPS = (((0, 1), "pe"), ((2, 3), "pe"), ((4,), "pe"), ((5,), "pe"))
MM_CHUNK = 512
MERGED_EX = True

_NC_CACHE = {}


def _build(widths=TILE_WIDTHS, groups=GROUPS):
    from concourse import bacc, mybir
    from concourse.tile import TileContext

    n_tiles = len(widths)
    fpl = sum(widths)
    n_groups = len(groups)
    offs = [sum(widths[:j]) for j in range(n_tiles + 1)]

    f32 = mybir.dt.float32
    bf16 = mybir.dt.bfloat16
    Alu = mybir.AluOpType
    Act = mybir.ActivationFunctionType

    nc = bacc.Bacc(trn_type="TRN2")
    big = nc.declare_dram_parameter("big", [P, 4 * fpl], bf16, isOutput=False)
    out = nc.declare_dram_parameter("out", [P, 3 * n_groups], f32, isOutput=True)
    out2 = nc.declare_dram_parameter("out2", [2, MM_CHUNK], f32, isOutput=True)

    n_mm_a = fpl // MM_CHUNK
    n_mm_d = sum(
        (offs[g[-1] + 1] - offs[g[0]]) // MM_CHUNK
        for g, dm in groups if dm == "pe"
    )
    has_pe_d = n_mm_d > 0

    with TileContext(nc) as tc:
        with (
            tc.tile_pool(name="io", bufs=5) as io_pool,
            tc.tile_pool(name="mid", bufs=3) as mid_pool,
            tc.tile_pool(name="acc", bufs=1) as acc_pool,
            tc.tile_pool(name="ps", bufs=1, space="PSUM") as psum_pool,
        ):
            # ex_full[:, 0, :] = e (= l+u-2t), ex_full[:, 1, :] = x (= l+u-2p)
            ex_full = acc_pool.tile([P, 2, fpl], bf16, tag="ex_full")
            wm_full = acc_pool.tile([P, fpl], bf16, tag="wm_full")
            max_span = max(offs[g[-1] + 1] - offs[g[0]] for g, _ in groups)
            junk = acc_pool.tile([P, max_span], bf16, tag="junk")
            acc_act = acc_pool.tile([P, 3 * n_groups], f32, tag="acc_act")
            ones = acc_pool.tile([P, 1], bf16, tag="ones")
            rowA = acc_pool.tile([1, MM_CHUNK], f32, tag="rowA")
            psA = psum_pool.tile([1, MM_CHUNK], f32, tag="psA")
            if has_pe_d:
                rd_full = acc_pool.tile([P, max_span], bf16, tag="rd_full")
                rowD = acc_pool.tile([1, MM_CHUNK], f32, tag="rowD")
                psD = psum_pool.tile([1, MM_CHUNK], f32, tag="psD")
            nc.vector.memset(ones, 1.0)

            mm_a = 0
            mm_d = 0

            for g_i, (tix, dm) in enumerate(groups):
                for j in tix:
                    fd = widths[j]
                    o = offs[j]
                    four = io_pool.tile([P, 4, fd], bf16, tag="four", name=f"four{j}")
                    src = big[:, 4 * o : 4 * o + 4 * fd].rearrange(
                        "p (s f) -> p s f", s=4
                    )
                    nc.sync.dma_start(out=four, in_=src)

                    lo = four[:, 0, :]
                    up = four[:, 1, :]

                    h = mid_pool.tile([P, fd], bf16, tag="h", name=f"h{j}")
                    nc.vector.tensor_add(out=h, in0=lo, in1=up)
                    nc.vector.tensor_sub(
                        out=wm_full[:, o : o + fd], in0=lo, in1=up
                    )
                    if MERGED_EX:
                        hb = h.unsqueeze(1).broadcast_to((P, 2, fd))
                        nc.vector.tensor_add(
                            out=ex_full[:, :, o : o + fd],
                            in0=four[:, 2:4, :], in1=hb,
                        )
                    else:
                        nc.vector.tensor_add(
                            out=ex_full[:, 0, o : o + fd], in0=h,
                            in1=four[:, 2, :],
                        )
                        nc.vector.tensor_add(
                            out=ex_full[:, 1, o : o + fd], in0=h,
                            in1=four[:, 3, :],
                        )
                    for c in range(fd // MM_CHUNK):
                        lo_c = o + c * MM_CHUNK
                        nc.tensor.matmul(
                            psA, ones, wm_full[:, lo_c : lo_c + MM_CHUNK],
                            start=(mm_a == 0), stop=(mm_a == n_mm_a - 1),
                        )
                        mm_a += 1

                # group-level ACT ops over the whole span
                go, ge = offs[tix[0]], offs[tix[-1] + 1]
                span = ge - go
                nc.scalar.activation(
                    out=junk[:, 0:span], in_=ex_full[:, 0, go:ge],
                    func=Act.Square, scale=0.5,
                    accum_out=acc_act[:, g_i : g_i + 1],
                )
                nc.scalar.activation(
                    out=junk[:, 0:span], in_=wm_full[:, go:ge],
                    func=Act.Relu,
                    accum_out=acc_act[:, n_groups + g_i : n_groups + g_i + 1],
                )
                if dm == "act":
                    nc.scalar.activation(
                        out=junk[:, 0:span], in_=ex_full[:, 1, go:ge],
                        func=Act.Relu, scale=0.5,
                        accum_out=acc_act[:, 2 * n_groups + g_i : 2 * n_groups + g_i + 1],
                    )
                else:
                    nc.vector.tensor_scalar(
                        out=rd_full[:, 0:span], in0=ex_full[:, 1, go:ge],
                        scalar1=0.5, scalar2=0.0,
                        op0=Alu.mult, op1=Alu.max,
                    )
                    for c in range(span // MM_CHUNK):
                        lo_c = c * MM_CHUNK
                        nc.tensor.matmul(
                            psD, ones, rd_full[:, lo_c : lo_c + MM_CHUNK],
                            start=(mm_d == 0), stop=(mm_d == n_mm_d - 1),
                        )
                        mm_d += 1

            nc.vector.tensor_copy(rowA, psA)
            nc.sync.dma_start(out=out2[0:1, :], in_=rowA)
            if has_pe_d:
                nc.vector.tensor_copy(rowD, psD)
                nc.sync.dma_start(out=out2[1:2, :], in_=rowD)
            nc.sync.dma_start(out=out[:, :], in_=acc_act)

    nc.compile()
    return nc


def _get_nc():
    key = (TILE_WIDTHS, GROUPS, MERGED_EX)
    if key not in _NC_CACHE:
        _NC_CACHE[key] = _build()
    return _NC_CACHE[key]


def _shard(inputs):
    import ml_dtypes

    bf = ml_dtypes.bfloat16
    pred = np.asarray(inputs["pred"], dtype=np.float32)
    targ = np.asarray(inputs["target"], dtype=np.float32).reshape(N)
    prev = np.asarray(inputs["prev_pci"], dtype=np.float32).reshape(N)
    pv = np.asarray(inputs["pv_values"]).reshape(N)

    lo = pred[:, 0]
    up = pred[:, 1]
    flip = pv != 0
    # v-fold: (l,u,t,p) -> (-u,-l,-t,-p) for pv==1 rows (exact sign encoding)
    lo2 = np.where(flip, -up, lo)
    up2 = np.where(flip, -lo, up)
    sg = np.where(flip, np.float32(2.0), np.float32(-2.0))
    lo_b = lo2.astype(bf)
    up_b = up2.astype(bf)
    t_b = (sg * targ).astype(bf)
    p_b = (sg * prev).astype(bf)

    in_maps = []
    for cix in range(N_CORES):
        s = slice(cix * NP_PER_CORE, (cix + 1) * NP_PER_CORE)
        streams = (
            lo_b[s].reshape(P, FPL),
            up_b[s].reshape(P, FPL),
            t_b[s].reshape(P, FPL),
            p_b[s].reshape(P, FPL),
        )
        parts = []
        off = 0
        for fd in TILE_WIDTHS:
            for st in streams:
                parts.append(st[:, off : off + fd])
            off += fd
        in_maps.append({"big": np.ascontiguousarray(np.concatenate(parts, axis=1))})
    return in_maps


def _combine(core_outs, core_outs2, n=N):
    n_groups = len(GROUPS)
    act_d = [g_i for g_i, (_, dm) in enumerate(GROUPS) if dm == "act"]
    B = C = D = A = 0.0
    for o, o2 in zip(core_outs, core_outs2):
        o = np.asarray(o, dtype=np.float64)
        o2 = np.asarray(o2, dtype=np.float64)
        B += o[:, 0:n_groups].sum()
        C += o[:, n_groups : 2 * n_groups].sum()
        D += o[:, [2 * n_groups + g for g in act_d]].sum()
        A += o2[0].sum()
        if len(act_d) < n_groups:
            D += o2[1].sum()
    center_loss = B / n
    width_loss = -A / n
    valid_penalty = C / n
    total = (
        center_loss * 10.0
        + 0.1 * width_loss
        + 10.0 * valid_penalty
        + 0.5 * D / n
    )
    return np.array(total, dtype=np.float32)


def _run(inputs, trace=False):
    from concourse.bass_utils import run_bass_kernel_spmd

    nc = _get_nc()
    in_maps = _shard(inputs)
    res = run_bass_kernel_spmd(
        nc, in_maps, core_ids=list(range(N_CORES)), trace=trace
    )
    core_outs = [res.results[c]["out"] for c in range(N_CORES)]
    core_outs2 = [res.results[c]["out2"] for c in range(N_CORES)]
    return _combine(core_outs, core_outs2), res


def kernel(**inputs) -> np.ndarray:
    result, _ = _run(inputs, trace=False)
    return result


# revision 11
# speedup vs baseline: 1.0063x; 1.0063x over previous
"""Trainium2 Bass kernel for the interval-prediction custom loss (v5).

total = 10*mean((t - c)^2) + 0.1*mean(u-l) + 10*mean(relu(l-u))
        + 0.5*sum(where(pv==0, relu(c-p), relu(p-c)))/N       with c=(l+u)/2

Host-side prep is layout/encoding only (shard, dtype cast, sign-fold):
  * v-fold: rows with pv==1 are re-encoded (l,u,t,p) -> (-u,-l,-t,-p).
    Exact and information-preserving: l-u and (t-c)^2 are invariant and the
    direction term relu(p-c) becomes relu(c'-p'), so the int64 pv stream
    (unsupported on device) disappears along with its select/mul ops.
  * bf16 streams per element: lo, up, tm2=-2t, pm2=-2p.

Device (per core, [128, 8192] elements), all DVE ops in 2x mode:
  per tile j (width fd): one DMA [P, 4, fd];
     h  = lo+up                       (TT)
     wm_full[j] = lo-up               (TT)
     ex_full[:, :, j] = tp + h_bcast  (ONE TT over [P,2,fd], h stride-0)
       -> e = h+tm2 = l+u-2t, x = h+pm2 = 2*sigma*(c-p)
  PE: psA += ones^T @ wm chunks (width loss partial sums, idle engine)
  per GROUP (amortizes the ~1.3us/op ACT fixed cost; trailing groups are
  single tiles in 'pe' mode so the post-DMA tail stays thin):
     ACT Square(e span, scale=.5) accum ; ACT Relu(wm span) accum
     'act': ACT Relu(x span, scale=.5) accum
     'pe' : DVE TS rd = max(.5x, 0) (4x) ; PE psD += ones^T @ rd chunks
Host combines group accumulators + the two PSUM rows in f64.
"""

import sys

if "/opt/trn_rl_repo" not in sys.path:
    sys.path.insert(0, "/opt/trn_rl_repo")

import numpy as np

N = 8388608
N_CORES = 8
P = 128
NP_PER_CORE = N // N_CORES            # 1048576
FPL = NP_PER_CORE // P                # 8192 elements per partition lane
TILE_WIDTHS = (512, 1536, 2048, 2048, 1536, 512)
assert sum(TILE_WIDTHS) == FPL
# Groups of consecutive tiles sharing one set of ACT ops; dmode 'act' or 'pe'
GROUPS = (((0, 1), "pe"), ((2,), "pe"), ((3,), "pe"), ((4,), "pe"), ((5,), "pe"))
MM_CHUNK = 512
MERGED_EX = True

_NC_CACHE = {}


def _build(widths=TILE_WIDTHS, groups=GROUPS):
    from concourse import bacc, mybir
    from concourse.tile import TileContext

    n_tiles = len(widths)
    fpl = sum(widths)
    n_groups = len(groups)
    offs = [sum(widths[:j]) for j in range(n_tiles + 1)]

    f32 = mybir.dt.float32
    bf16 = mybir.dt.bfloat16
    Alu = mybir.AluOpType
    Act = mybir.ActivationFunctionType

    nc = bacc.Bacc(trn_type="TRN2")
    big = nc.declare_dram_parameter("big", [P, 4 * fpl], bf16, isOutput=False)
    out = nc.declare_dram_parameter("out", [P, 3 * n_groups], f32, isOutput=True)
    out2 = nc.declare_dram_parameter("out2", [2, MM_CHUNK], f32, isOutput=True)

    n_mm_a = fpl // MM_CHUNK
    n_mm_d = sum(
        (offs[g[-1] + 1] - offs[g[0]]) // MM_CHUNK
        for g, dm in groups if dm == "pe"
    )
    has_pe_d = n_mm_d > 0

    with TileContext(nc) as tc:
        with (
            tc.tile_pool(name="io", bufs=5) as io_pool,
            tc.tile_pool(name="mid", bufs=3) as mid_pool,
            tc.tile_pool(name="acc", bufs=1) as acc_pool,
            tc.tile_pool(name="ps", bufs=1, space="PSUM") as psum_pool,
        ):
            # ex_full[:, 0, :] = e (= l+u-2t), ex_full[:, 1, :] = x (= l+u-2p)
            ex_full = acc_pool.tile([P, 2, fpl], bf16, tag="ex_full")
            wm_full = acc_pool.tile([P, fpl], bf16, tag="wm_full")
            max_span = max(offs[g[-1] + 1] - offs[g[0]] for g, _ in groups)
            junk = acc_pool.tile([P, max_span], bf16, tag="junk")
            acc_act = acc_pool.tile([P, 3 * n_groups], f32, tag="acc_act")
            ones = acc_pool.tile([P, 1], bf16, tag="ones")
            rowA = acc_pool.tile([1, MM_CHUNK], f32, tag="rowA")
            psA = psum_pool.tile([1, MM_CHUNK], f32, tag="psA")
            if has_pe_d:
                rd_full = acc_pool.tile([P, max_span], bf16, tag="rd_full")
                rowD = acc_pool.tile([1, MM_CHUNK], f32, tag="rowD")
                psD = psum_pool.tile([1, MM_CHUNK], f32, tag="psD")
            nc.vector.memset(ones, 1.0)

            mm_a = 0
            mm_d = 0

            for g_i, (tix, dm) in enumerate(groups):
                for j in tix:
                    fd = widths[j]
                    o = offs[j]
                    four = io_pool.tile([P, 4, fd], bf16, tag="four", name=f"four{j}")
                    src = big[:, 4 * o : 4 * o + 4 * fd].rearrange(
                        "p (s f) -> p s f", s=4
                    )
                    nc.sync.dma_start(out=four, in_=src)

                    lo = four[:, 0, :]
                    up = four[:, 1, :]

                    h = mid_pool.tile([P, fd], bf16, tag="h", name=f"h{j}")
                    nc.vector.tensor_add(out=h, in0=lo, in1=up)
                    nc.vector.tensor_sub(
                        out=wm_full[:, o : o + fd], in0=lo, in1=up
                    )
                    if MERGED_EX:
                        hb = h.unsqueeze(1).broadcast_to((P, 2, fd))
                        nc.vector.tensor_add(
                            out=ex_full[:, :, o : o + fd],
                            in0=four[:, 2:4, :], in1=hb,
                        )
                    else:
                        nc.vector.tensor_add(
                            out=ex_full[:, 0, o : o + fd], in0=h,
                            in1=four[:, 2, :],
                        )
                        nc.vector.tensor_add(
                            out=ex_full[:, 1, o : o + fd], in0=h,
                            in1=four[:, 3, :],
                        )
                    for c in range(fd // MM_CHUNK):
                        lo_c = o + c * MM_CHUNK
                        nc.tensor.matmul(
                            psA, ones, wm_full[:, lo_c : lo_c + MM_CHUNK],
                            start=(mm_a == 0), stop=(mm_a == n_mm_a - 1),
                        )
                        mm_a += 1

                # group-level ACT ops over the whole span
                go, ge = offs[tix[0]], offs[tix[-1] + 1]
                span = ge - go
                nc.scalar.activation(
                    out=junk[:, 0:span], in_=ex_full[:, 0, go:ge],
                    func=Act.Square, scale=0.5,
                    accum_out=acc_act[:, g_i : g_i + 1],
                )
                nc.scalar.activation(
                    out=junk[:, 0:span], in_=wm_full[:, go:ge],
                    func=Act.Relu,
                    accum_out=acc_act[:, n_groups + g_i : n_groups + g_i + 1],
                )
                if dm == "act":
                    nc.scalar.activation(
                        out=junk[:, 0:span], in_=ex_full[:, 1, go:ge],
                        func=Act.Relu, scale=0.5,
                        accum_out=acc_act[:, 2 * n_groups + g_i : 2 * n_groups + g_i + 1],
                    )
                else:
                    nc.vector.tensor_scalar(
                        out=rd_full[:, 0:span], in0=ex_full[:, 1, go:ge],
                        scalar1=0.5, scalar2=0.0,
                        op0=Alu.mult, op1=Alu.max,
                    )
                    for c in range(span // MM_CHUNK):
                        lo_c = c * MM_CHUNK
                        nc.tensor.matmul(
                            psD, ones, rd_full[:, lo_c : lo_c + MM_CHUNK],
                            start=(mm_d == 0), stop=(mm_d == n_mm_d - 1),
                        )
                        mm_d += 1

            nc.vector.tensor_copy(rowA, psA)
            nc.sync.dma_start(out=out2[0:1, :], in_=rowA)
            if has_pe_d:
                nc.vector.tensor_copy(rowD, psD)
                nc.sync.dma_start(out=out2[1:2, :], in_=rowD)
            nc.sync.dma_start(out=out[:, :], in_=acc_act)

    nc.compile()
    return nc


def _get_nc():
    key = (TILE_WIDTHS, GROUPS, MERGED_EX)
    if key not in _NC_CACHE:
        _NC_CACHE[key] = _build()
    return _NC_CACHE[key]


def _shard(inputs):
    import ml_dtypes

    bf = ml_dtypes.bfloat16
    pred = np.asarray(inputs["pred"], dtype=np.float32)
    targ = np.asarray(inputs["target"], dtype=np.float32).reshape(N)
    prev = np.asarray(inputs["prev_pci"], dtype=np.float32).reshape(N)
    pv = np.asarray(inputs["pv_values"]).reshape(N)

    lo = pred[:, 0]
    up = pred[:, 1]
    flip = pv != 0
    # v-fold: (l,u,t,p) -> (-u,-l,-t,-p) for pv==1 rows (exact sign encoding)
    lo2 = np.where(flip, -up, lo)
    up2 = np.where(flip, -lo, up)
    sg = np.where(flip, np.float32(2.0), np.float32(-2.0))
    lo_b = lo2.astype(bf)
    up_b = up2.astype(bf)
    t_b = (sg * targ).astype(bf)
    p_b = (sg * prev).astype(bf)

    in_maps = []
    for cix in range(N_CORES):
        s = slice(cix * NP_PER_CORE, (cix + 1) * NP_PER_CORE)
        streams = (
            lo_b[s].reshape(P, FPL),
            up_b[s].reshape(P, FPL),
            t_b[s].reshape(P, FPL),
            p_b[s].reshape(P, FPL),
        )
        parts = []
        off = 0
        for fd in TILE_WIDTHS:
            for st in streams:
                parts.append(st[:, off : off + fd])
            off += fd
        in_maps.append({"big": np.ascontiguousarray(np.concatenate(parts, axis=1))})
    return in_maps


def _combine(core_outs, core_outs2, n=N):
    n_groups = len(GROUPS)
    act_d = [g_i for g_i, (_, dm) in enumerate(GROUPS) if dm == "act"]
    B = C = D = A = 0.0
    for o, o2 in zip(core_outs, core_outs2):
        o = np.asarray(o, dtype=np.float64)
        o2 = np.asarray(o2, dtype=np.float64)
        B += o[:, 0:n_groups].sum()
        C += o[:, n_groups : 2 * n_groups].sum()
        D += o[:, [2 * n_groups + g for g in act_d]].sum()
        A += o2[0].sum()
        if len(act_d) < n_groups:
            D += o2[1].sum()
    center_loss = B / n
    width_loss = -A / n
    valid_penalty = C / n
    total = (
        center_loss * 10.0
        + 0.1 * width_loss
        + 10.0 * valid_penalty
        + 0.5 * D / n
    )
    return np.array(total, dtype=np.float32)


def _run(inputs, trace=False):
    from concourse.bass_utils import run_bass_kernel_spmd

    nc = _get_nc()
    in_maps = _shard(inputs)
    res = run_bass_kernel_spmd(
        nc, in_maps, core_ids=list(range(N_CORES)), trace=trace
    )
    core_outs = [res.results[c]["out"] for c in range(N_CORES)]
    core_outs2 = [res.results[c]["out2"] for c in range(N_CORES)]
    return _combine(core_outs, core_outs2), res


def kernel(**inputs) -> np.ndarray:
    result, _ = _run(inputs, trace=False)
    return result
